# revision 39
# baseline (speedup 1.0000x reference)
"""Trainium2 (Bass) SPMD kernel for the CBGNN message-passing problem. v5.

Structure (per core, 8-way SPMD):
  A: per-cycle MLP scores for its 32768 cycles -> 1MB AllGather (out_tab).
  B: seg0-sorted edge stream [128 x 4224]: indirect-gather out_tab[seg1],
     exp-weighted masked scans -> segment softmax numerator/denominator,
     extracted at segment-end slots -> o2 (+64 shift) -> 1MB AllGather
     (out2_tab).
  C: target-aligned row-balanced Edge2cycle stream: indirect-gather
     out2_tab[src], masked max-scan -> per-target maxima in LOCAL DRAM.
     Per-target extraction happens locally (no 17MB AllGather as in v3).
  D: full MLP2 for the core's own nonempty targets (om extracted locally);
     closed-form fast path for empty targets (om == 0, whether_k >= 0);
     global L2 norm via 16B AllReduce; sigmoid.

v5 vs v3: phase C extraction is local (credM AllGather removed, phase-D
om gather from shared memory removed); MLP inner ops are grouped 4
sub-blocks per instruction (2-bank PSUM tiles + windowed tensor_reduce)
to cut instruction count ~3x.
"""

import sys

import numpy as np

for _p in ("/opt/trn_rl_repo",):
    if _p not in sys.path:
        sys.path.insert(0, _p)

NCORES = 8
P = 128
ELEM = 64


class Cfg:
    n_cyc = 262144
    out_dim = 256
    e_cc = 4194304
    m_e2c = 4194304
    len_edges = 1048576
    t1 = 4224            # phase-B stream columns per partition row
    t2 = 4224            # phase-C stream columns per partition row
    mcols = 528          # gather chunk width (stream columns)
    a_mac = 2048         # phase-A macro tile (tokens)
    tokd_full = 36864    # phase-D full-MLP capacity per core
    d_chunk = 9216       # phase-D featT chunk (tokens)
    neg_slope = 0.2
    ln_eps = 1e-5
    cshift = 64.0        # positivity shift for the max-scan
    mlp_bf16 = True
    tab2_bf16 = False    # out2_tab stays f32: +64-shifted values need mantissa
    act_lrelu = True     # False: DVE max(x, ax) (CoreSim lacks Lrelu)
    phases = "abcd"

    @property
    def seg_pc(self):
        return self.n_cyc // NCORES

    @property
    def toka(self):
        return self.n_cyc // NCORES

    @property
    def tokd(self):
        return self.len_edges // NCORES


class SmallCfg(Cfg):
    n_cyc = 16384
    e_cc = 65536
    m_e2c = 65536
    len_edges = 16384
    t1 = 96
    t2 = 96
    mcols = 48
    a_mac = 2048
    tokd_full = 2560
    d_chunk = 2560


# ---------------------------------------------------------------------------
# host-side sharding / layout (index work + parameter folding only)
# ---------------------------------------------------------------------------

def _pack_rows(counts, ncols):
    csum = np.cumsum(counts)
    nseg = len(counts)
    seg_row = np.empty(nseg, np.int64)
    seg_col0 = np.empty(nseg, np.int64)
    start = 0
    base = 0
    for r in range(P):
        j = int(np.searchsorted(csum, base + ncols, side="right"))
        if j < nseg and counts[j] > ncols:
            raise ValueError("segment larger than a row")
        prev = base
        seg_row[start:j] = r
        seg_col0[start:j] = (csum[start:j] - counts[start:j]) - prev
        if j > 0:
            base = int(csum[j - 1])
        start = j
        if start == nseg:
            break
    if start != nseg:
        raise ValueError("edges did not fit into P rows")
    return seg_row, seg_col0


def _layout_stream(named_vals, seg_local, seg_counts, seg_starts_local, ncols):
    seg_row, seg_col0 = _pack_rows(seg_counts, ncols)
    rank = np.arange(len(seg_local)) - seg_starts_local[seg_local]
    slot = seg_row[seg_local] * ncols + seg_col0[seg_local] + rank
    out = {}
    for name, (vals, fill) in named_vals.items():
        arr = np.full(P * ncols, fill, dtype=np.asarray(vals).dtype)
        arr[slot] = vals
        out[name] = arr.reshape(P, ncols)
    lab = np.full(P * ncols, -1, np.int64)
    lab[slot] = seg_local
    lab2 = lab.reshape(P, ncols)
    msk = np.zeros((P, ncols), np.float32)
    msk[:, 1:] = ((lab2[:, 1:] == lab2[:, :-1]) & (lab2[:, 1:] >= 0)).astype(
        np.float32)
    out["__mask"] = msk
    end_slot = seg_row * ncols + seg_col0 + seg_counts - 1
    filler_slot = P * ncols - 1
    assert lab.reshape(-1)[filler_slot] == -1, "last slot is not filler"
    out["__end"] = np.where(seg_counts > 0, end_slot,
                            filler_slot).astype(np.int64)
    return out


def _fold_mlp(W1, g, b, W2, b2):
    W2 = np.asarray(W2, np.float64).reshape(-1)
    g = np.asarray(g, np.float64)
    b = np.asarray(b, np.float64)
    w2eff = g * W2
    b2eff = float(np.asarray(b2).reshape(-1)[0]) + float(np.dot(b, W2))
    return (w2eff.astype(np.float32), np.float32(b2eff),
            np.float32(w2eff.sum()))


def host_prepare(inputs, cfg):
    n_cyc, seg_pc = cfg.n_cyc, cfg.seg_pc
    od = cfg.out_dim

    x = np.asarray(inputs["x"], np.float32)
    e2c = np.asarray(inputs["Edge2cycle"])
    eidx = np.asarray(inputs["edge_index"])
    pce = np.asarray(inputs["permuteCE"], np.float32)
    wk = np.asarray(inputs["whether_k"], np.float32)
    assert float(wk.min()) >= 0.0, "fast path requires whether_k >= 0"

    seg0 = np.asarray(eidx[0], np.int64)
    seg1 = np.asarray(eidx[1], np.int64)

    xT = np.ascontiguousarray(x.T)
    spr = cfg.toka // P

    def tab_pos(g):
        core = g // seg_pc
        loc = g - core * seg_pc
        return core * seg_pc + (loc % P) * spr + loc // P

    order0 = np.argsort(seg0, kind="stable")
    seg0s = seg0[order0]
    cnt0 = np.bincount(seg0, minlength=n_cyc).astype(np.int64)
    start0 = np.zeros(n_cyc + 1, np.int64)
    np.cumsum(cnt0, out=start0[1:])

    tgt = np.asarray(e2c[:, 0], np.int64)
    src = np.asarray(e2c[:, 1], np.int64)
    order1 = np.argsort(tgt, kind="stable")
    tgts = tgt[order1]
    cnt1 = np.bincount(tgt, minlength=cfg.len_edges).astype(np.int64)
    start1 = np.zeros(cfg.len_edges + 1, np.int64)
    np.cumsum(cnt1, out=start1[1:])

    # ---- parameter folding
    w2eff, b2e, s2 = _fold_mlp(inputs["W1"], inputs["g1"], inputs["b1"],
                               inputs["W2"], inputs["b2"])
    wk2eff, bk2e, s2k = _fold_mlp(inputs["Wk1"], inputs["gk"], inputs["bk"],
                                  inputs["Wk2"], inputs["bk2"])
    w1_h = np.asarray(inputs["W1"], np.float32)
    wk1_h = np.asarray(inputs["Wk1"], np.float32)
    # fast-path constants (om == 0): h = wk * leaky(Wk1[1]) for wk >= 0
    lb = np.where(wk1_h[1] > 0, wk1_h[1], cfg.neg_slope * wk1_h[1]).astype(
        np.float64)
    m0 = lb.mean()
    v0 = (lb ** 2).mean() - m0 ** 2
    d0 = float(np.dot(lb, wk2eff.astype(np.float64)))
    g0 = d0 - m0 * float(s2k)

    cst = np.zeros(16, np.float32)
    cst[0], cst[1], cst[2], cst[3] = -s2, b2e, -s2k, bk2e
    cst[4] = np.float32(cfg.ln_eps)
    cst[5] = np.float32(v0)
    cst[6] = np.float32(g0)
    # wpack layout: [0:od] W1[0:128] | [od:2od] W1[128:256] | [2od:3od] w2eff
    # | [3od:4od] wk2eff | [4od:4od+16] cst | [4od+16:5od+16] Wk1 (rows 0-1)
    wcols = 5 * od + 16
    wpack = np.zeros((P, wcols), np.float32)
    wpack[:, 0:od] = w1_h[0:P]
    wpack[:, od:2 * od] = w1_h[P:2 * P]
    wpack[:, 2 * od:3 * od] = np.broadcast_to(w2eff, (P, od))
    wpack[:, 3 * od:4 * od] = np.broadcast_to(wk2eff, (P, od))
    wpack[:, 4 * od:4 * od + 16] = np.broadcast_to(cst, (P, 16))
    wpack[0:2, 4 * od + 16:5 * od + 16] = wk1_h

    # ---- phase C sharding: contiguous target ranges, row-balanced
    tot1 = int(start1[-1])
    tgb = [0]
    for c in range(1, NCORES):
        tgb.append(int(np.searchsorted(start1, tot1 * c // NCORES)))
    tgb.append(cfg.len_edges)

    # empty targets: evenly sharded fast path
    T_e = np.nonzero(cnt1 == 0)[0]
    per_e = -(-len(T_e) // NCORES) if len(T_e) else 0
    assert per_e <= cfg.tokd, (per_e, cfg.tokd)

    bcols = seg_pc // P
    fcols = cfg.tokd_full // P
    ecols = cfg.tokd // P
    sentinel_slot = P * cfg.t2 - 1

    in_maps = []
    tne_list = []
    for c in range(NCORES):
        m = {}
        m["xT"] = np.ascontiguousarray(
            xT[:, c * cfg.toka:(c + 1) * cfg.toka]).astype(np.float16)
        m["wpack"] = wpack

        # --- phase B stream
        lo, hi = c * seg_pc, (c + 1) * seg_pc
        glo, ghi = int(start0[lo]), int(start0[hi])
        e_sel = order0[glo:ghi]
        segl = (seg0s[glo:ghi] - lo)
        scounts = cnt0[lo:hi]
        sstarts = (start0[lo:hi] - glo)
        pos1 = tab_pos(seg1[e_sel]).astype(np.int64)
        st = _layout_stream(
            {"bp": (pce[e_sel], np.float32(-300.0)),
             "bpos": (pos1, np.int64(n_cyc))},
            segl, scounts, sstarts, cfg.t1)
        m["bp"] = st["bp"]
        m["bmsk"] = st["__mask"]
        m["bpos"] = st["bpos"].astype(np.int32)
        m["bidx"] = st["__end"].reshape(P, bcols).astype(np.int32)

        # --- phase C stream (own target range)
        lo1, hi1 = tgb[c], tgb[c + 1]
        g1lo, g1hi = int(start1[lo1]), int(start1[hi1])
        r_sel = order1[g1lo:g1hi]
        tgtl = (tgts[g1lo:g1hi] - lo1)
        tcounts = cnt1[lo1:hi1]
        tstarts = (start1[lo1:hi1] - g1lo)
        st2 = _layout_stream(
            {"cpos": (src[r_sel].astype(np.int64), np.int64(n_cyc))},
            tgtl, tcounts, tstarts, cfg.t2)
        m["cmsk"] = st2["__mask"]
        m["cpos"] = st2["cpos"].astype(np.int32)

        # --- phase D full path: own nonempty targets, local extraction
        ne = tcounts > 0
        tne = np.nonzero(ne)[0] + lo1
        nf = len(tne)
        assert nf <= cfg.tokd_full, (nf, cfg.tokd_full)
        tne_list.append(tne)
        cidxF = np.full(cfg.tokd_full, sentinel_slot, np.int64)
        cidxF[:nf] = st2["__end"][ne]
        m["cidxf"] = cidxF.reshape(P, fcols).astype(np.int32)
        wkF = np.zeros(cfg.tokd_full, np.float32)
        wkF[:nf] = wk[tne]
        m["wkf"] = wkF
        mF = np.zeros(cfg.tokd_full, np.float32)
        mF[:nf] = 1.0
        # yF lives in matmul slot order: token ch*d_chunk + sub*128 + p sits
        # at (p, ch*dsub + sub); lay the mask out to match
        nch = cfg.tokd_full // cfg.d_chunk
        dsub = cfg.d_chunk // P
        m["mf"] = np.ascontiguousarray(
            mF.reshape(nch, dsub, P).transpose(2, 0, 1).reshape(
                P, cfg.tokd_full // P))

        # --- phase D fast path (empty targets, even shard)
        te = T_e[c * per_e:(c + 1) * per_e]
        nE = len(te)
        wkE = np.zeros(cfg.tokd, np.float32)
        wkE[:nE] = wk[te]
        m["wke"] = wkE.reshape(P, ecols)
        mE = np.zeros(cfg.tokd, np.float32)
        mE[:nE] = 1.0
        m["me"] = mE.reshape(P, ecols)
        in_maps.append(m)

    asm = {"tne": tne_list, "T_e": T_e, "per_e": per_e}
    return in_maps, asm


def assemble_output(results, asm, cfg):
    T_e, per_e = asm["T_e"], asm["per_e"]
    nch = cfg.tokd_full // cfg.d_chunk
    subs = cfg.d_chunk // P
    out = np.empty(cfg.len_edges, np.float32)
    for c in range(NCORES):
        yf = np.asarray(results[c]["y"]).reshape(P, nch, subs)
        yf = yf.transpose(1, 2, 0).reshape(-1)
        tne = asm["tne"][c]
        out[tne] = yf[:len(tne)]
        ye = np.asarray(results[c]["y2"]).reshape(-1)
        te = T_e[c * per_e:(c + 1) * per_e]
        out[te] = ye[:len(te)]
    return out


# ---------------------------------------------------------------------------
# device program
# ---------------------------------------------------------------------------

def build_nc(cfg):
    import concourse.bass as bass
    import concourse.bacc as bacc
    import concourse.mybir as mybir
    import concourse.tile as tile
    from contextlib import ExitStack

    dt = mybir.dt
    f32, i32, bf16 = dt.float32, dt.int32, dt.float16
    hdt = bf16 if cfg.mlp_bf16 else f32
    Alu = mybir.AluOpType
    Act = mybir.ActivationFunctionType

    n_cyc, od = cfg.n_cyc, cfg.out_dim
    seg_pc = cfg.seg_pc
    toka, tokd, tokdf = cfg.toka, cfg.tokd, cfg.tokd_full
    t1, t2, mcols = cfg.t1, cfg.t2, cfg.mcols
    kch = od // P
    spr = toka // P
    bcols = seg_pc // P
    fcols = tokdf // P
    ecols = tokd // P
    nch_b = t1 // mcols
    nch_c = t2 // mcols
    groups = [list(range(NCORES))]

    nc = bacc.Bacc(trn_type="TRN2", num_devices=NCORES)

    def din(name, shape, dtype=f32):
        if dtype is None:
            dtype = bf16
        return nc.declare_dram_parameter(name, list(shape), dtype, False).ap()

    wcols = 5 * od + 16
    xT = din("xT", [od, toka], None)
    wpack = din("wpack", [P, wcols])
    bp = din("bp", [P, t1])
    bmsk = din("bmsk", [P, t1])
    bpos = din("bpos", [P, t1], i32)
    bidx = din("bidx", [P, bcols], i32)
    cmsk = din("cmsk", [P, t2])
    cpos = din("cpos", [P, t2], i32)
    cidxf = din("cidxf", [P, fcols], i32)
    wkf = din("wkf", [tokdf])
    mf = din("mf", [P, fcols])
    wke = din("wke", [P, ecols])
    me = din("me", [P, ecols])
    y_out = nc.declare_dram_parameter("y", [tokdf], f32, True).ap()
    y2_out = nc.declare_dram_parameter("y2", [tokd], f32, True).ap()

    out_part = nc.dram_tensor("out_part", [toka], bf16).ap()
    out_tab = nc.dram_tensor("out_tab", [n_cyc + ELEM], bf16,
                             addr_space="Shared").ap()
    bredW = nc.dram_tensor("bredW", [P * t1], f32).ap()
    bredU = nc.dram_tensor("bredU", [P * t1], f32).ap()
    t2dt = bf16 if cfg.tab2_bf16 else f32
    out2_part = nc.dram_tensor("out2_part", [seg_pc], t2dt).ap()
    out2_tab = nc.dram_tensor("out2_tab", [n_cyc + ELEM], t2dt,
                              addr_space="Shared").ap()
    credM = nc.dram_tensor("credM", [P * t2], f32).ap()
    featT = nc.dram_tensor("featT", [2, tokdf], f32).ap()
    nsq_part = nc.dram_tensor("nsq_part", [16], f32).ap()
    nsq_tab = nc.dram_tensor("nsq_tab", [16], f32, addr_space="Shared").ap()

    def r2(ap_, p=P):
        return ap_.rearrange("(p c) -> p c", p=p)

    def col(ap_):
        return ap_.rearrange("(a b) -> a b", b=1)

    def _finish(ctx):
        ctx.close()
        return nc

    with ExitStack() as ctx:
        tc = ctx.enter_context(tile.TileContext(nc))
        cpool = ctx.enter_context(tc.tile_pool(name="cpool", bufs=1))
        sb = ctx.enter_context(tc.tile_pool(name="sb", bufs=2))
        sb3 = ctx.enter_context(tc.tile_pool(name="sb3", bufs=3))
        ps = ctx.enter_context(tc.tile_pool(name="ps", bufs=3, space="PSUM"))
        ps1 = ctx.enter_context(tc.tile_pool(name="ps1", bufs=1,
                                             space="PSUM"))

        def stt(out, in0, scalar, in1, op0, op1, accum=None):
            nc.vector.scalar_tensor_tensor(out=out, in0=in0, scalar=scalar,
                                           in1=in1, op0=op0, op1=op1,
                                           accum_out=accum)

        from concourse.tile import add_dep_helper as _adh
        loose = []

        def DMA(*a, **kw):
            inst = nc.sync.dma_start(*a, **kw)
            loose.append(inst)
            return inst

        def IDMA(*a, **kw):
            inst = nc.gpsimd.indirect_dma_start(*a, **kw)
            loose.append(inst)
            return inst

        def CC(*a, **kw):
            inst = nc.gpsimd.collective_compute(*a, **kw)
            loose.append(inst)
            return inst

        def fence():
            items = list(loose)
            loose.clear()
            if not items:
                return
            for eng in (nc.vector, nc.scalar, nc.tensor, nc.gpsimd,
                        nc.sync):
                for j in range(0, len(items), 2):
                    nop = eng.nop()
                    for d in items[j:j + 2]:
                        _adh(nop.ins, d.ins, sync=True, reason="fence")
            tc.no_sync_barrier()

        # ---- constants: one DMA
        wp = cpool.tile([P, wcols], f32, tag="wp")
        DMA(out=wp[:], in_=wpack[:, :])
        w1b_sb = cpool.tile([P, kch * od], bf16, tag="w1b")
        for k in range(kch):
            nc.vector.tensor_copy(out=w1b_sb[:, k * od:(k + 1) * od],
                                  in_=wp[:, k * od:(k + 1) * od])
        w1_sb = [w1b_sb[:, k * od:(k + 1) * od] for k in range(kch)]
        cst = wp[:, 4 * od:4 * od + 16]
        wk1_sb = wp[0:2, 4 * od + 16:5 * od + 16]
        w2b_sb = cpool.tile([P, 4 * od], hdt, tag="w2b")
        for q in range(4):
            nc.vector.tensor_copy(out=w2b_sb[:, q * od:(q + 1) * od],
                                  in_=wp[:, 2 * od:3 * od])
        wk2b_sb = cpool.tile([P, 4 * od], hdt, tag="wk2b")
        for q in range(4):
            nc.vector.tensor_copy(out=wk2b_sb[:, q * od:(q + 1) * od],
                                  in_=wp[:, 3 * od:4 * od])
        sent0 = cpool.tile([1, ELEM], bf16, tag="sent0")
        nc.gpsimd.memset(sent0[:], 0.0)
        sent1 = cpool.tile([1, ELEM], t2dt, tag="sent1")
        nc.gpsimd.memset(sent1[:], -60000.0)

        # shared MLP->scalar block: nsub 128-token sub-blocks, grouped 4
        # per 2-bank PSUM tile to cut instruction count.
        def mlp_block(nsub, lhsT_for, neg_s2_col, b2e_col, w2rep_tile,
                      out_cols):
            assert nsub % 4 == 0
            S = sb.tile([P, nsub], f32, tag="mlpS")
            Q = sb.tile([P, nsub], f32, tag="mlpQ")
            D_ = sb.tile([P, nsub], f32, tag="mlpD")
            for g in range(nsub // 4):
                pst = ps.tile([P, 4 * od], f32, tag="mlp_ps")
                for q in range(4):
                    s = g * 4 + q
                    pieces = lhsT_for(s)
                    for i, (lt, wt) in enumerate(pieces):
                        nc.tensor.matmul(out=pst[:, q * od:(q + 1) * od],
                                         lhsT=lt, rhs=wt,
                                         start=(i == 0),
                                         stop=(i == len(pieces) - 1))
                h = sb3.tile([P, 4 * od], hdt, tag="mlp_h")
                if cfg.act_lrelu:
                    nc.scalar.activation(out=h[:], in_=pst[:],
                                         func=Act.Lrelu,
                                         alpha=cfg.neg_slope)
                else:
                    stt(h[:], pst[:], cfg.neg_slope, pst[:], Alu.mult,
                        Alu.max)
                nc.vector.tensor_reduce(
                    out=S[:, g * 4:(g + 1) * 4],
                    in_=h[:].rearrange("p (q e) -> p q e", e=od),
                    axis=mybir.AxisListType.X, op=Alu.add)
                hsq = sb3.tile([P, 4 * od], hdt, tag="mlp_hsq")
                stt(hsq[:], h[:], 1.0, h[:], Alu.mult, Alu.mult)
                nc.vector.tensor_reduce(
                    out=Q[:, g * 4:(g + 1) * 4],
                    in_=hsq[:].rearrange("p (q e) -> p q e", e=od),
                    axis=mybir.AxisListType.X, op=Alu.add)
                dsc = sb3.tile([P, 4 * od], hdt, tag="mlp_dsc")
                stt(dsc[:], h[:], 1.0, w2rep_tile[:], Alu.mult, Alu.mult)
                nc.vector.tensor_reduce(
                    out=D_[:, g * 4:(g + 1) * 4],
                    in_=dsc[:].rearrange("p (q e) -> p q e", e=od),
                    axis=mybir.AxisListType.X, op=Alu.add)
            mu = sb.tile([P, nsub], f32, tag="mlp_mu")
            nc.vector.tensor_scalar_mul(out=mu[:], in0=S[:],
                                        scalar1=1.0 / od)
            var = sb.tile([P, nsub], f32, tag="mlp_var")
            stt(var[:], mu[:], 1.0, mu[:], Alu.mult, Alu.mult)
            qn = sb.tile([P, nsub], f32, tag="mlp_qn")
            nc.vector.tensor_scalar_mul(out=qn[:], in0=Q[:],
                                        scalar1=1.0 / od)
            nc.vector.tensor_tensor(out=var[:], in0=qn[:], in1=var[:],
                                    op=Alu.subtract)
            sd = sb.tile([P, nsub], f32, tag="mlp_sd")
            nc.scalar.activation(out=sd[:], in_=var[:], func=Act.Sqrt,
                                 bias=cst[:, 4:5])
            rs = sb.tile([P, nsub], f32, tag="mlp_rs")
            nc.vector.reciprocal(out=rs[:], in_=sd[:])
            tmp = sb.tile([P, nsub], f32, tag="mlp_tmp")
            stt(tmp[:], mu[:], neg_s2_col, D_[:], Alu.mult, Alu.add)
            nc.vector.tensor_tensor(out=tmp[:], in0=tmp[:], in1=rs[:],
                                    op=Alu.mult)
            stt(out_cols, tmp[:], 1.0, b2e_col.to_broadcast([P, nsub]),
                Alu.mult, Alu.add)

        # chunked stream gather: [P, w] of table[pos]
        def gather_chunk(pool, tab_col, pos_dram_sl, w, tag, vdt=bf16):
            pos_sb = pool.tile([P, w], i32, tag=tag + "_pos")
            DMA(out=pos_sb[:], in_=pos_dram_sl)
            val = pool.tile([P, w], vdt, tag=tag + "_val")
            IDMA(out=val[:], out_offset=None, in_=tab_col,
                 in_offset=bass.IndirectOffsetOnAxis(ap=pos_sb[:], axis=0))
            return val

        fence()

        # ============================================================
        # Phase A
        # ============================================================
        out_sb = cpool.tile([P, spr], bf16, tag="out_sb")
        nmac = toka // cfg.a_mac
        msub = cfg.a_mac // P
        xTk = xT.rearrange("(k p) c -> p k c", k=kch)
        with tc.tile_pool(name="apool", bufs=2) as apl:
            for mblk in range(nmac):
                xt = apl.tile([P, kch * cfg.a_mac], bf16, tag="xt")
                DMA(
                    out=xt[:],
                    in_=xTk[:, :, mblk * cfg.a_mac:(mblk + 1) * cfg.a_mac])

                def lhsT_a(s, _xt=xt):
                    return [(_xt[:, k * cfg.a_mac + s * P:
                                 k * cfg.a_mac + (s + 1) * P], w1_sb[k])
                            for k in range(kch)]

                mlp_block(msub, lhsT_a, cst[:, 0:1], cst[:, 1:2], w2b_sb,
                          out_sb[:, mblk * msub:(mblk + 1) * msub])
        DMA(out=r2(out_part), in_=out_sb[:])

        CC("AllGather", Alu.bypass, replica_groups=groups,
           ins=[out_part[:]], outs=[out_tab[0:n_cyc]])
        DMA(out=r2(out_tab[n_cyc:n_cyc + ELEM], p=1), in_=sent0[:])
        fence()

        def stub_outputs():
            dumf = cpool.tile([P, fcols], f32, tag="dumf")
            nc.gpsimd.memset(dumf[:], 0.0)
            dume = cpool.tile([P, ecols], f32, tag="dume")
            nc.gpsimd.memset(dume[:], 0.0)
            DMA(out=r2(y_out), in_=dumf[:])
            DMA(out=r2(y2_out), in_=dume[:])

        if "b" not in cfg.phases:
            stub_outputs()
            return _finish(ctx)

        # ============================================================
        # Phase B
        # ============================================================
        with tc.tile_pool(name="bpool", bufs=2) as bpl, \
                tc.tile_pool(name="bstage", bufs=1) as bst:
            wstage = bst.tile([P, t1], f32, tag="wstage")
            ustage = bst.tile([P, t1], f32, tag="ustage")
            for i in range(nch_b):
                sl = slice(i * mcols, (i + 1) * mcols)
                val = gather_chunk(bpl, col(out_tab[:]), bpos[:, sl],
                                   mcols, "bg")
                bpc = bpl.tile([P, mcols], f32, tag="bpc")
                DMA(out=bpc[:], in_=bp[:, sl])
                bmc = bpl.tile([P, mcols], f32, tag="bmc")
                DMA(out=bmc[:], in_=bmsk[:, sl])
                stt(bpc[:], bpc[:], cfg.neg_slope, bpc[:], Alu.mult,
                    Alu.max)
                nc.scalar.activation(out=bpc[:], in_=bpc[:], func=Act.Exp)
                uval = bpl.tile([P, mcols], f32, tag="uval")
                nc.vector.tensor_tensor(out=uval[:], in0=bpc[:],
                                        in1=val[:], op=Alu.mult)
                nc.vector.tensor_tensor_scan(
                    out=wstage[:, sl], data0=bmc[:], data1=bpc[:],
                    initial=(0.0 if i == 0 else
                             wstage[:, i * mcols - 1:i * mcols]),
                    op0=Alu.mult, op1=Alu.add)
                nc.vector.tensor_tensor_scan(
                    out=ustage[:, sl], data0=bmc[:], data1=uval[:],
                    initial=(0.0 if i == 0 else
                             ustage[:, i * mcols - 1:i * mcols]),
                    op0=Alu.mult, op1=Alu.add)
            DMA(out=r2(bredW), in_=wstage[:])
            DMA(out=r2(bredU), in_=ustage[:])
            fence()

        # ---- segment extraction: dW/dU at end slots, then o2
        bidx_sb = cpool.tile([P, bcols], i32, tag="bidx_sb")
        DMA(out=bidx_sb[:], in_=bidx[:, :])
        dW = cpool.tile([P, bcols], f32, tag="dW")
        IDMA(out=dW[:], out_offset=None, in_=col(bredW[:]),
             in_offset=bass.IndirectOffsetOnAxis(ap=bidx_sb[:], axis=0))
        dU = cpool.tile([P, bcols], f32, tag="dU")
        IDMA(out=dU[:], out_offset=None, in_=col(bredU[:]),
             in_offset=bass.IndirectOffsetOnAxis(ap=bidx_sb[:], axis=0))
        o2 = cpool.tile([P, bcols], f32, tag="o2")
        nc.vector.tensor_scalar_add(out=o2[:], in0=dW[:], scalar1=1e-30)
        nc.vector.reciprocal(out=o2[:], in_=o2[:])
        nc.vector.tensor_tensor(out=o2[:], in0=o2[:], in1=dU[:],
                                op=Alu.mult)
        nc.vector.tensor_scalar_add(out=o2[:], in0=o2[:],
                                    scalar1=cfg.cshift)
        DMA(out=r2(out2_part), in_=o2[:])
        fence()
        CC("AllGather", Alu.bypass, replica_groups=groups,
           ins=[out2_part[:]], outs=[out2_tab[0:n_cyc]])
        DMA(out=r2(out2_tab[n_cyc:n_cyc + ELEM], p=1), in_=sent1[:])
        fence()

        if "c" not in cfg.phases:
            stub_outputs()
            return _finish(ctx)

        # ============================================================
        # Phase C
        # ============================================================
        with tc.tile_pool(name="cpool2", bufs=2) as cpl, \
                tc.tile_pool(name="cstage", bufs=1) as cstg:
            mstage = cstg.tile([P, t2], f32, tag="mstage")
            for i in range(nch_c):
                sl = slice(i * mcols, (i + 1) * mcols)
                val = gather_chunk(cpl, col(out2_tab[:]), cpos[:, sl],
                                   mcols, "cg", vdt=t2dt)
                cmc = cpl.tile([P, mcols], f32, tag="cmc")
                DMA(out=cmc[:], in_=cmsk[:, sl])
                if cfg.tab2_bf16:
                    valf = cpl.tile([P, mcols], f32, tag="valf")
                    nc.vector.tensor_copy(out=valf[:], in_=val[:])
                else:
                    valf = val
                nc.vector.tensor_tensor_scan(
                    out=mstage[:, sl], data0=cmc[:], data1=valf[:],
                    initial=(0.0 if i == 0 else
                             mstage[:, i * mcols - 1:i * mcols]),
                    op0=Alu.mult, op1=Alu.max)
            DMA(out=r2(credM), in_=mstage[:])
            fence()

        # ---- per-target max extraction (LOCAL) + om -> featT
        cidx_sb = cpool.tile([P, fcols], i32, tag="cidx_sb")
        DMA(out=cidx_sb[:], in_=cidxf[:, :])
        omr = cpool.tile([P, fcols], f32, tag="omr")
        IDMA(out=omr[:], out_offset=None, in_=col(credM[:]),
             in_offset=bass.IndirectOffsetOnAxis(ap=cidx_sb[:], axis=0))
        omm = cpool.tile([P, fcols], f32, tag="omm")
        nc.vector.tensor_scalar(out=omm[:], in0=omr[:], scalar1=32.0,
                                scalar2=None, op0=Alu.is_gt)
        om = cpool.tile([P, fcols], f32, tag="om")
        stt(om[:], omr[:], -cfg.cshift, omm[:], Alu.add, Alu.mult)
        DMA(out=r2(featT[0, :]), in_=om[:])
        wkb = cpool.tile([P, fcols], f32, tag="wkb")
        DMA(out=wkb[:], in_=r2(wkf))
        DMA(out=r2(featT[1, :]), in_=wkb[:])
        fence()

        if "d" not in cfg.phases:
            stub_outputs()
            return _finish(ctx)

        # ============================================================
        # Phase D: full MLP on own nonempty targets
        # ============================================================
        yF = cpool.tile([P, fcols], f32, tag="yF")
        nchunk = tokdf // cfg.d_chunk
        dsub = cfg.d_chunk // P
        with tc.tile_pool(name="dpool", bufs=2) as dpl:
            for ch in range(nchunk):
                ft = dpl.tile([2, cfg.d_chunk], f32, tag="ft")
                DMA(
                    out=ft[:],
                    in_=featT[:, ch * cfg.d_chunk:(ch + 1) * cfg.d_chunk])

                def lhsT_d(s, _ft=ft):
                    return [(_ft[:, s * P:(s + 1) * P], wk1_sb)]

                mlp_block(dsub, lhsT_d, cst[:, 2:3], cst[:, 3:4], wk2b_sb,
                          yF[:, ch * dsub:(ch + 1) * dsub])

        fence()

        # ---- fast path: empty targets (om == 0)
        wke_sb = cpool.tile([P, ecols], f32, tag="wke_sb")
        DMA(out=wke_sb[:], in_=wke[:, :])
        wk2t = cpool.tile([P, ecols], f32, tag="wk2t")
        stt(wk2t[:], wke_sb[:], 1.0, wke_sb[:], Alu.mult, Alu.mult)
        nc.vector.tensor_scalar(out=wk2t[:], in0=wk2t[:],
                                scalar1=cst[:, 5:6], scalar2=cst[:, 4:5],
                                op0=Alu.mult, op1=Alu.add)
        nc.scalar.activation(out=wk2t[:], in_=wk2t[:], func=Act.Sqrt)
        nc.vector.reciprocal(out=wk2t[:], in_=wk2t[:])
        yE = cpool.tile([P, ecols], f32, tag="yE")
        nc.vector.tensor_scalar(out=yE[:], in0=wke_sb[:],
                                scalar1=cst[:, 6:7], scalar2=None,
                                op0=Alu.mult)
        nc.vector.tensor_tensor(out=yE[:], in0=yE[:], in1=wk2t[:],
                                op=Alu.mult)
        nc.vector.tensor_scalar(out=yE[:], in0=yE[:], scalar1=cst[:, 3:4],
                                scalar2=None, op0=Alu.add)

        # ---- global L2 norm
        mf_sb = cpool.tile([P, fcols], f32, tag="mf_sb")
        DMA(out=mf_sb[:], in_=mf[:, :])
        me_sb = cpool.tile([P, ecols], f32, tag="me_sb")
        DMA(out=me_sb[:], in_=me[:, :])
        ssq = cpool.tile([P, 2], f32, tag="ssq")
        scrF = cpool.tile([P, fcols], f32, tag="scrF")
        nc.vector.tensor_tensor(out=scrF[:], in0=yF[:], in1=mf_sb[:],
                                op=Alu.mult)
        scrF2 = cpool.tile([P, fcols], f32, tag="scrF2")
        stt(scrF2[:], scrF[:], 1.0, yF[:], Alu.mult, Alu.mult,
            accum=ssq[:, 0:1])
        scrE = cpool.tile([P, ecols], f32, tag="scrE")
        nc.vector.tensor_tensor(out=scrE[:], in0=yE[:], in1=me_sb[:],
                                op=Alu.mult)
        scrE2 = cpool.tile([P, ecols], f32, tag="scrE2")
        stt(scrE2[:], scrE[:], 1.0, yE[:], Alu.mult, Alu.mult,
            accum=ssq[:, 1:2])
        ssqt = cpool.tile([P, 1], f32, tag="ssqt")
        nc.vector.tensor_tensor(out=ssqt[:], in0=ssq[:, 0:1],
                                in1=ssq[:, 1:2], op=Alu.add)
        ones = cpool.tile([P, 1], f32, tag="ones")
        nc.gpsimd.memset(ones[:], 1.0)
        sred = ps1.tile([1, 1], f32, tag="sred")
        nc.tensor.matmul(out=sred[:], lhsT=ones[:], rhs=ssqt[:],
                         start=True, stop=True)
        nsq_sb = cpool.tile([1, 16], f32, tag="nsq_sb")
        nc.gpsimd.memset(nsq_sb[:], 0.0)
        nc.vector.tensor_copy(out=nsq_sb[:, 0:1], in_=sred[:])
        DMA(out=r2(nsq_part, p=1), in_=nsq_sb[:])
        fence()
        CC("AllReduce", Alu.add, replica_groups=groups,
           ins=[nsq_part[:]], outs=[nsq_tab[:]])
        fence()
        nrm = cpool.tile([1, 1], f32, tag="nrm")
        DMA(out=nrm[:], in_=r2(nsq_tab[0:1], p=1))
        nc.scalar.activation(out=nrm[:], in_=nrm[:], func=Act.Sqrt)
        nc.vector.tensor_scalar_max(out=nrm[:], in0=nrm[:], scalar1=1e-12)
        nc.vector.reciprocal(out=nrm[:], in_=nrm[:])
        ones_row = cpool.tile([1, P], f32, tag="ones_row")
        nc.gpsimd.memset(ones_row[:], 1.0)
        rn_ps = ps1.tile([P, 1], f32, tag="rn_ps")
        nc.tensor.matmul(out=rn_ps[:], lhsT=ones_row[:], rhs=nrm[:],
                         start=True, stop=True)
        rn_sb = cpool.tile([P, 1], f32, tag="rn_sb")
        nc.vector.tensor_copy(out=rn_sb[:], in_=rn_ps[:])
        # sigmoid(x) = 1/(1+exp(-x)) via Exp + HW reciprocal
        def scale_sigmoid(t):
            nc.scalar.activation(out=t, in_=t, func=Act.Exp,
                                 scale=nrn_sb[:, 0:1])
            nc.vector.tensor_scalar_add(out=t, in0=t, scalar1=1.0)
            nc.vector.reciprocal(out=t, in_=t)

        nrn_sb = cpool.tile([P, 1], f32, tag="nrn_sb")
        nc.vector.tensor_scalar_mul(out=nrn_sb[:], in0=rn_sb[:],
                                    scalar1=-1.0)
        scale_sigmoid(yF[:])
        DMA(out=r2(y_out), in_=yF[:])
        scale_sigmoid(yE[:])
        DMA(out=r2(y2_out), in_=yE[:])

    return nc


# ---------------------------------------------------------------------------
# entry point
# ---------------------------------------------------------------------------

_NC_CACHE = {}


def _get_nc(cfg):
    key = (cfg.n_cyc, cfg.e_cc, cfg.len_edges, cfg.t1, cfg.t2,
           cfg.tokd_full, cfg.phases)
    if key not in _NC_CACHE:
        nc = build_nc(cfg)
        if not nc.is_finalized():
            nc.finalize()
        _NC_CACHE[key] = nc
    return _NC_CACHE[key]


def run(inputs, cfg=None, trace=False):
    from concourse.bass_utils import run_bass_kernel_spmd
    cfg = cfg or Cfg()
    in_maps, asm = host_prepare(inputs, cfg)
    nc = _get_nc(cfg)
    res = run_bass_kernel_spmd(nc, in_maps, core_ids=list(range(NCORES)),
                               trace=trace)
    return assemble_output(res.results, asm, cfg), res


def kernel(**inputs):
    out, _ = run(inputs)
    return out


# revision 41
# speedup vs baseline: 1.3312x; 1.3312x over previous
"""Trainium2 (Bass) SPMD kernel for the CBGNN message-passing problem. v5.

Structure (per core, 8-way SPMD):
  A: per-cycle MLP scores for its 32768 cycles -> 1MB AllGather (out_tab).
  B: seg0-sorted edge stream [128 x 4224]: indirect-gather out_tab[seg1],
     exp-weighted masked scans -> segment softmax numerator/denominator,
     extracted at segment-end slots -> o2 (+64 shift) -> 1MB AllGather
     (out2_tab).
  C: target-aligned row-balanced Edge2cycle stream: indirect-gather
     out2_tab[src], masked max-scan -> per-target maxima in LOCAL DRAM.
     Per-target extraction happens locally (no 17MB AllGather as in v3).
  D: full MLP2 for the core's own nonempty targets (om extracted locally);
     closed-form fast path for empty targets (om == 0, whether_k >= 0);
     global L2 norm via 16B AllReduce; sigmoid.

v5 vs v3: phase C extraction is local (credM AllGather removed, phase-D
om gather from shared memory removed); MLP inner ops are grouped 4
sub-blocks per instruction (2-bank PSUM tiles + windowed tensor_reduce)
to cut instruction count ~3x.
"""

import sys

import numpy as np

for _p in ("/opt/trn_rl_repo",):
    if _p not in sys.path:
        sys.path.insert(0, _p)

NCORES = 8
P = 128
ELEM = 64


class Cfg:
    n_cyc = 262144
    out_dim = 256
    e_cc = 4194304
    m_e2c = 4194304
    len_edges = 1048576
    t1 = 4224            # phase-B stream columns per partition row
    t2 = 4224            # phase-C stream columns per partition row
    mcols = 528          # gather chunk width (stream columns)
    a_mac = 2048         # phase-A macro tile (tokens)
    tokd_full = 36864    # phase-D full-MLP capacity per core
    d_chunk = 9216       # phase-D featT chunk (tokens)
    neg_slope = 0.2
    ln_eps = 1e-5
    cshift = 64.0        # positivity shift for the max-scan
    mlp_bf16 = True
    tab2_bf16 = False    # out2_tab stays f32: +64-shifted values need mantissa
    act_lrelu = True     # False: DVE max(x, ax) (CoreSim lacks Lrelu)
    phases = "abcd"

    @property
    def seg_pc(self):
        return self.n_cyc // NCORES

    @property
    def toka(self):
        return self.n_cyc // NCORES

    @property
    def tokd(self):
        return self.len_edges // NCORES


class SmallCfg(Cfg):
    n_cyc = 16384
    e_cc = 65536
    m_e2c = 65536
    len_edges = 16384
    t1 = 96
    t2 = 96
    mcols = 48
    a_mac = 2048
    tokd_full = 2560
    d_chunk = 2560


# ---------------------------------------------------------------------------
# host-side sharding / layout (index work + parameter folding only)
# ---------------------------------------------------------------------------

def _pack_rows(counts, ncols):
    csum = np.cumsum(counts)
    nseg = len(counts)
    seg_row = np.empty(nseg, np.int64)
    seg_col0 = np.empty(nseg, np.int64)
    start = 0
    base = 0
    for r in range(P):
        j = int(np.searchsorted(csum, base + ncols, side="right"))
        if j < nseg and counts[j] > ncols:
            raise ValueError("segment larger than a row")
        prev = base
        seg_row[start:j] = r
        seg_col0[start:j] = (csum[start:j] - counts[start:j]) - prev
        if j > 0:
            base = int(csum[j - 1])
        start = j
        if start == nseg:
            break
    if start != nseg:
        raise ValueError("edges did not fit into P rows")
    return seg_row, seg_col0


def _layout_stream(named_vals, seg_local, seg_counts, seg_starts_local, ncols):
    seg_row, seg_col0 = _pack_rows(seg_counts, ncols)
    rank = np.arange(len(seg_local)) - seg_starts_local[seg_local]
    slot = seg_row[seg_local] * ncols + seg_col0[seg_local] + rank
    out = {}
    for name, (vals, fill) in named_vals.items():
        arr = np.full(P * ncols, fill, dtype=np.asarray(vals).dtype)
        arr[slot] = vals
        out[name] = arr.reshape(P, ncols)
    lab = np.full(P * ncols, -1, np.int64)
    lab[slot] = seg_local
    lab2 = lab.reshape(P, ncols)
    msk = np.zeros((P, ncols), np.float32)
    msk[:, 1:] = ((lab2[:, 1:] == lab2[:, :-1]) & (lab2[:, 1:] >= 0)).astype(
        np.float32)
    out["__mask"] = msk
    end_slot = seg_row * ncols + seg_col0 + seg_counts - 1
    filler_slot = P * ncols - 1
    assert lab.reshape(-1)[filler_slot] == -1, "last slot is not filler"
    out["__end"] = np.where(seg_counts > 0, end_slot,
                            filler_slot).astype(np.int64)
    return out


def _fold_mlp(W1, g, b, W2, b2):
    W2 = np.asarray(W2, np.float64).reshape(-1)
    g = np.asarray(g, np.float64)
    b = np.asarray(b, np.float64)
    w2eff = g * W2
    b2eff = float(np.asarray(b2).reshape(-1)[0]) + float(np.dot(b, W2))
    return (w2eff.astype(np.float32), np.float32(b2eff),
            np.float32(w2eff.sum()))


def host_prepare(inputs, cfg):
    n_cyc, seg_pc = cfg.n_cyc, cfg.seg_pc
    od = cfg.out_dim

    x = np.asarray(inputs["x"], np.float32)
    e2c = np.asarray(inputs["Edge2cycle"])
    eidx = np.asarray(inputs["edge_index"])
    pce = np.asarray(inputs["permuteCE"], np.float32)
    wk = np.asarray(inputs["whether_k"], np.float32)
    assert float(wk.min()) >= 0.0, "fast path requires whether_k >= 0"

    seg0 = np.asarray(eidx[0], np.int64)
    seg1 = np.asarray(eidx[1], np.int64)

    xT = np.ascontiguousarray(x.T)
    spr = cfg.toka // P

    def tab_pos(g):
        core = g // seg_pc
        loc = g - core * seg_pc
        return core * seg_pc + (loc % P) * spr + loc // P

    order0 = np.argsort(seg0, kind="stable")
    seg0s = seg0[order0]
    cnt0 = np.bincount(seg0, minlength=n_cyc).astype(np.int64)
    start0 = np.zeros(n_cyc + 1, np.int64)
    np.cumsum(cnt0, out=start0[1:])

    tgt = np.asarray(e2c[:, 0], np.int64)
    src = np.asarray(e2c[:, 1], np.int64)
    order1 = np.argsort(tgt, kind="stable")
    tgts = tgt[order1]
    cnt1 = np.bincount(tgt, minlength=cfg.len_edges).astype(np.int64)
    start1 = np.zeros(cfg.len_edges + 1, np.int64)
    np.cumsum(cnt1, out=start1[1:])

    # ---- parameter folding
    w2eff, b2e, s2 = _fold_mlp(inputs["W1"], inputs["g1"], inputs["b1"],
                               inputs["W2"], inputs["b2"])
    wk2eff, bk2e, s2k = _fold_mlp(inputs["Wk1"], inputs["gk"], inputs["bk"],
                                  inputs["Wk2"], inputs["bk2"])
    w1_h = np.asarray(inputs["W1"], np.float32)
    wk1_h = np.asarray(inputs["Wk1"], np.float32)
    # fast-path constants (om == 0): h = wk * leaky(Wk1[1]) for wk >= 0
    lb = np.where(wk1_h[1] > 0, wk1_h[1], cfg.neg_slope * wk1_h[1]).astype(
        np.float64)
    m0 = lb.mean()
    v0 = (lb ** 2).mean() - m0 ** 2
    d0 = float(np.dot(lb, wk2eff.astype(np.float64)))
    g0 = d0 - m0 * float(s2k)

    cst = np.zeros(16, np.float32)
    cst[0], cst[1], cst[2], cst[3] = -s2, b2e, -s2k, bk2e
    cst[4] = np.float32(cfg.ln_eps)
    cst[5] = np.float32(v0)
    cst[6] = np.float32(g0)
    # wpack layout: [0:od] W1[0:128] | [od:2od] W1[128:256] | [2od:3od] w2eff
    # | [3od:4od] wk2eff | [4od:4od+16] cst | [4od+16:5od+16] Wk1 (rows 0-1)
    wcols = 5 * od + 16
    wpack = np.zeros((P, wcols), np.float32)
    wpack[:, 0:od] = w1_h[0:P]
    wpack[:, od:2 * od] = w1_h[P:2 * P]
    wpack[:, 2 * od:3 * od] = np.broadcast_to(w2eff, (P, od))
    wpack[:, 3 * od:4 * od] = np.broadcast_to(wk2eff, (P, od))
    wpack[:, 4 * od:4 * od + 16] = np.broadcast_to(cst, (P, 16))
    wpack[0:2, 4 * od + 16:5 * od + 16] = wk1_h

    # ---- phase C sharding: contiguous target ranges, row-balanced
    tot1 = int(start1[-1])
    tgb = [0]
    for c in range(1, NCORES):
        tgb.append(int(np.searchsorted(start1, tot1 * c // NCORES)))
    tgb.append(cfg.len_edges)

    # empty targets: evenly sharded fast path
    T_e = np.nonzero(cnt1 == 0)[0]
    per_e = -(-len(T_e) // NCORES) if len(T_e) else 0
    assert per_e <= cfg.tokd, (per_e, cfg.tokd)

    bcols = seg_pc // P
    fcols = cfg.tokd_full // P
    ecols = cfg.tokd // P
    sentinel_slot = P * cfg.t2 - 1

    in_maps = []
    tne_list = []
    for c in range(NCORES):
        m = {}
        m["xT"] = np.ascontiguousarray(
            xT[:, c * cfg.toka:(c + 1) * cfg.toka]).astype(np.float16)
        m["wpack"] = wpack

        # --- phase B stream
        lo, hi = c * seg_pc, (c + 1) * seg_pc
        glo, ghi = int(start0[lo]), int(start0[hi])
        e_sel = order0[glo:ghi]
        segl = (seg0s[glo:ghi] - lo)
        scounts = cnt0[lo:hi]
        sstarts = (start0[lo:hi] - glo)
        pos1 = tab_pos(seg1[e_sel]).astype(np.int64)
        st = _layout_stream(
            {"bp": (pce[e_sel], np.float32(-300.0)),
             "bpos": (pos1, np.int64(n_cyc))},
            segl, scounts, sstarts, cfg.t1)
        m["bp"] = st["bp"]
        m["bmsk"] = st["__mask"]
        m["bpos"] = st["bpos"].astype(np.int32)
        m["bidx"] = st["__end"].reshape(P, bcols).astype(np.int32)

        # --- phase C stream (own target range)
        lo1, hi1 = tgb[c], tgb[c + 1]
        g1lo, g1hi = int(start1[lo1]), int(start1[hi1])
        r_sel = order1[g1lo:g1hi]
        tgtl = (tgts[g1lo:g1hi] - lo1)
        tcounts = cnt1[lo1:hi1]
        tstarts = (start1[lo1:hi1] - g1lo)
        st2 = _layout_stream(
            {"cpos": (src[r_sel].astype(np.int64), np.int64(n_cyc))},
            tgtl, tcounts, tstarts, cfg.t2)
        m["cmsk"] = st2["__mask"]
        m["cpos"] = st2["cpos"].astype(np.int32)

        # --- phase D full path: own nonempty targets, local extraction
        ne = tcounts > 0
        tne = np.nonzero(ne)[0] + lo1
        nf = len(tne)
        assert nf <= cfg.tokd_full, (nf, cfg.tokd_full)
        tne_list.append(tne)
        cidxF = np.full(cfg.tokd_full, sentinel_slot, np.int64)
        cidxF[:nf] = st2["__end"][ne]
        m["cidxf"] = cidxF.reshape(P, fcols).astype(np.int32)
        wkF = np.zeros(cfg.tokd_full, np.float32)
        wkF[:nf] = wk[tne]
        m["wkf"] = wkF
        mF = np.zeros(cfg.tokd_full, np.float32)
        mF[:nf] = 1.0
        # yF lives in matmul slot order: token ch*d_chunk + sub*128 + p sits
        # at (p, ch*dsub + sub); lay the mask out to match
        nch = cfg.tokd_full // cfg.d_chunk
        dsub = cfg.d_chunk // P
        m["mf"] = np.ascontiguousarray(
            mF.reshape(nch, dsub, P).transpose(2, 0, 1).reshape(
                P, cfg.tokd_full // P))

        # --- phase D fast path (empty targets, even shard)
        te = T_e[c * per_e:(c + 1) * per_e]
        nE = len(te)
        wkE = np.zeros(cfg.tokd, np.float32)
        wkE[:nE] = wk[te]
        m["wke"] = wkE.reshape(P, ecols)
        mE = np.zeros(cfg.tokd, np.float32)
        mE[:nE] = 1.0
        m["me"] = mE.reshape(P, ecols)
        in_maps.append(m)

    asm = {"tne": tne_list, "T_e": T_e, "per_e": per_e}
    return in_maps, asm


def assemble_output(results, asm, cfg):
    T_e, per_e = asm["T_e"], asm["per_e"]
    nch = cfg.tokd_full // cfg.d_chunk
    subs = cfg.d_chunk // P
    out = np.empty(cfg.len_edges, np.float32)
    for c in range(NCORES):
        yf = np.asarray(results[c]["y"]).reshape(P, nch, subs)
        yf = yf.transpose(1, 2, 0).reshape(-1)
        tne = asm["tne"][c]
        out[tne] = yf[:len(tne)]
        ye = np.asarray(results[c]["y2"]).reshape(-1)
        te = T_e[c * per_e:(c + 1) * per_e]
        out[te] = ye[:len(te)]
    return out


# ---------------------------------------------------------------------------
# device program
# ---------------------------------------------------------------------------

def build_nc(cfg):
    import concourse.bass as bass
    import concourse.bacc as bacc
    import concourse.mybir as mybir
    import concourse.tile as tile
    from contextlib import ExitStack

    dt = mybir.dt
    f32, i32, bf16 = dt.float32, dt.int32, dt.float16
    hdt = bf16 if cfg.mlp_bf16 else f32
    Alu = mybir.AluOpType
    Act = mybir.ActivationFunctionType

    n_cyc, od = cfg.n_cyc, cfg.out_dim
    seg_pc = cfg.seg_pc
    toka, tokd, tokdf = cfg.toka, cfg.tokd, cfg.tokd_full
    t1, t2, mcols = cfg.t1, cfg.t2, cfg.mcols
    kch = od // P
    spr = toka // P
    bcols = seg_pc // P
    fcols = tokdf // P
    ecols = tokd // P
    nch_b = t1 // mcols
    nch_c = t2 // mcols
    groups = [list(range(NCORES))]

    nc = bacc.Bacc(trn_type="TRN2", num_devices=NCORES)

    def din(name, shape, dtype=f32):
        if dtype is None:
            dtype = bf16
        return nc.declare_dram_parameter(name, list(shape), dtype, False).ap()

    wcols = 5 * od + 16
    xT = din("xT", [od, toka], None)
    wpack = din("wpack", [P, wcols])
    bp = din("bp", [P, t1])
    bmsk = din("bmsk", [P, t1])
    bpos = din("bpos", [P, t1], i32)
    bidx = din("bidx", [P, bcols], i32)
    cmsk = din("cmsk", [P, t2])
    cpos = din("cpos", [P, t2], i32)
    cidxf = din("cidxf", [P, fcols], i32)
    wkf = din("wkf", [tokdf])
    mf = din("mf", [P, fcols])
    wke = din("wke", [P, ecols])
    me = din("me", [P, ecols])
    y_out = nc.declare_dram_parameter("y", [tokdf], f32, True).ap()
    y2_out = nc.declare_dram_parameter("y2", [tokd], f32, True).ap()

    out_part = nc.dram_tensor("out_part", [toka], bf16).ap()
    out_tab = nc.dram_tensor("out_tab", [n_cyc + ELEM], bf16,
                             addr_space="Shared").ap()
    bredW = nc.dram_tensor("bredW", [P * t1], f32).ap()
    bredU = nc.dram_tensor("bredU", [P * t1], f32).ap()
    t2dt = bf16 if cfg.tab2_bf16 else f32
    out2_part = nc.dram_tensor("out2_part", [seg_pc], t2dt).ap()
    out2_tab = nc.dram_tensor("out2_tab", [n_cyc + ELEM], t2dt,
                              addr_space="Shared").ap()
    credM = nc.dram_tensor("credM", [P * t2], f32).ap()
    featT = nc.dram_tensor("featT", [2, tokdf], f32).ap()
    nsq_part = nc.dram_tensor("nsq_part", [16], f32).ap()
    nsq_tab = nc.dram_tensor("nsq_tab", [16], f32, addr_space="Shared").ap()

    def r2(ap_, p=P):
        return ap_.rearrange("(p c) -> p c", p=p)

    def col(ap_):
        return ap_.rearrange("(a b) -> a b", b=1)

    def _finish(ctx):
        ctx.close()
        return nc

    with ExitStack() as ctx:
        tc = ctx.enter_context(tile.TileContext(nc))
        cpool = ctx.enter_context(tc.tile_pool(name="cpool", bufs=1))
        sb = ctx.enter_context(tc.tile_pool(name="sb", bufs=2))
        sb3 = ctx.enter_context(tc.tile_pool(name="sb3", bufs=3))
        ps = ctx.enter_context(tc.tile_pool(name="ps", bufs=3, space="PSUM"))
        ps1 = ctx.enter_context(tc.tile_pool(name="ps1", bufs=1,
                                             space="PSUM"))

        def stt(out, in0, scalar, in1, op0, op1, accum=None):
            nc.vector.scalar_tensor_tensor(out=out, in0=in0, scalar=scalar,
                                           in1=in1, op0=op0, op1=op1,
                                           accum_out=accum)

        from concourse.tile import add_dep_helper as _adh
        loose = []

        def DMA(*a, **kw):
            inst = nc.sync.dma_start(*a, **kw)
            loose.append(inst)
            return inst

        def IDMA(*a, **kw):
            inst = nc.gpsimd.indirect_dma_start(*a, **kw)
            loose.append(inst)
            return inst

        def CC(*a, **kw):
            inst = nc.gpsimd.collective_compute(*a, **kw)
            loose.append(inst)
            return inst

        def fence():
            items = list(loose)
            loose.clear()
            if not items:
                return
            for eng in (nc.vector, nc.scalar, nc.tensor, nc.gpsimd,
                        nc.sync):
                for j in range(0, len(items), 2):
                    nop = eng.nop()
                    for d in items[j:j + 2]:
                        _adh(nop.ins, d.ins, sync=True, reason="fence")
            tc.no_sync_barrier()

        # ---- constants: one DMA
        wp = cpool.tile([P, wcols], f32, tag="wp")
        DMA(out=wp[:], in_=wpack[:, :])
        w1b_sb = cpool.tile([P, kch * od], bf16, tag="w1b")
        for k in range(kch):
            nc.vector.tensor_copy(out=w1b_sb[:, k * od:(k + 1) * od],
                                  in_=wp[:, k * od:(k + 1) * od])
        w1_sb = [w1b_sb[:, k * od:(k + 1) * od] for k in range(kch)]
        cst = wp[:, 4 * od:4 * od + 16]
        wk1_sb = wp[0:2, 4 * od + 16:5 * od + 16]
        w2b_sb = cpool.tile([P, 4 * od], hdt, tag="w2b")
        for q in range(4):
            nc.vector.tensor_copy(out=w2b_sb[:, q * od:(q + 1) * od],
                                  in_=wp[:, 2 * od:3 * od])
        wk2b_sb = cpool.tile([P, 4 * od], hdt, tag="wk2b")
        for q in range(4):
            nc.vector.tensor_copy(out=wk2b_sb[:, q * od:(q + 1) * od],
                                  in_=wp[:, 3 * od:4 * od])
        sent0 = cpool.tile([1, ELEM], bf16, tag="sent0")
        nc.gpsimd.memset(sent0[:], 0.0)
        sent1 = cpool.tile([1, ELEM], t2dt, tag="sent1")
        nc.gpsimd.memset(sent1[:], -60000.0)

        # shared MLP->scalar block: nsub 128-token sub-blocks, grouped 4
        # per 2-bank PSUM tile to cut instruction count.
        def mlp_block(nsub, lhsT_for, neg_s2_col, b2e_col, w2rep_tile,
                      out_cols):
            assert nsub % 4 == 0
            S = sb.tile([P, nsub], f32, tag="mlpS")
            Q = sb.tile([P, nsub], f32, tag="mlpQ")
            D_ = sb.tile([P, nsub], f32, tag="mlpD")
            for g in range(nsub // 4):
                pst = ps.tile([P, 4 * od], f32, tag="mlp_ps")
                for q in range(4):
                    s = g * 4 + q
                    pieces = lhsT_for(s)
                    for i, (lt, wt) in enumerate(pieces):
                        nc.tensor.matmul(out=pst[:, q * od:(q + 1) * od],
                                         lhsT=lt, rhs=wt,
                                         start=(i == 0),
                                         stop=(i == len(pieces) - 1))
                h = sb3.tile([P, 4 * od], hdt, tag="mlp_h")
                if cfg.act_lrelu:
                    nc.scalar.activation(out=h[:], in_=pst[:],
                                         func=Act.Lrelu,
                                         alpha=cfg.neg_slope)
                else:
                    stt(h[:], pst[:], cfg.neg_slope, pst[:], Alu.mult,
                        Alu.max)
                nc.vector.tensor_reduce(
                    out=S[:, g * 4:(g + 1) * 4],
                    in_=h[:].rearrange("p (q e) -> p q e", e=od),
                    axis=mybir.AxisListType.X, op=Alu.add)
                hsq = sb3.tile([P, 4 * od], hdt, tag="mlp_hsq")
                stt(hsq[:], h[:], 1.0, h[:], Alu.mult, Alu.mult)
                nc.vector.tensor_reduce(
                    out=Q[:, g * 4:(g + 1) * 4],
                    in_=hsq[:].rearrange("p (q e) -> p q e", e=od),
                    axis=mybir.AxisListType.X, op=Alu.add)
                dsc = sb3.tile([P, 4 * od], hdt, tag="mlp_dsc")
                stt(dsc[:], h[:], 1.0, w2rep_tile[:], Alu.mult, Alu.mult)
                nc.vector.tensor_reduce(
                    out=D_[:, g * 4:(g + 1) * 4],
                    in_=dsc[:].rearrange("p (q e) -> p q e", e=od),
                    axis=mybir.AxisListType.X, op=Alu.add)
            mu = sb.tile([P, nsub], f32, tag="mlp_mu")
            nc.vector.tensor_scalar_mul(out=mu[:], in0=S[:],
                                        scalar1=1.0 / od)
            var = sb.tile([P, nsub], f32, tag="mlp_var")
            stt(var[:], mu[:], 1.0, mu[:], Alu.mult, Alu.mult)
            qn = sb.tile([P, nsub], f32, tag="mlp_qn")
            nc.vector.tensor_scalar_mul(out=qn[:], in0=Q[:],
                                        scalar1=1.0 / od)
            nc.vector.tensor_tensor(out=var[:], in0=qn[:], in1=var[:],
                                    op=Alu.subtract)
            sd = sb.tile([P, nsub], f32, tag="mlp_sd")
            nc.scalar.activation(out=sd[:], in_=var[:], func=Act.Sqrt,
                                 bias=cst[:, 4:5])
            rs = sb.tile([P, nsub], f32, tag="mlp_rs")
            nc.vector.reciprocal(out=rs[:], in_=sd[:])
            tmp = sb.tile([P, nsub], f32, tag="mlp_tmp")
            stt(tmp[:], mu[:], neg_s2_col, D_[:], Alu.mult, Alu.add)
            nc.vector.tensor_tensor(out=tmp[:], in0=tmp[:], in1=rs[:],
                                    op=Alu.mult)
            stt(out_cols, tmp[:], 1.0, b2e_col.to_broadcast([P, nsub]),
                Alu.mult, Alu.add)

        # chunked stream gather: [P, w] of table[pos]
        def gather_chunk(pool, tab_col, pos_dram_sl, w, tag, vdt=bf16):
            pos_sb = pool.tile([P, w], i32, tag=tag + "_pos")
            DMA(out=pos_sb[:], in_=pos_dram_sl)
            val = pool.tile([P, w], vdt, tag=tag + "_val")
            IDMA(out=val[:], out_offset=None, in_=tab_col,
                 in_offset=bass.IndirectOffsetOnAxis(ap=pos_sb[:], axis=0))
            return val

        fence()

        # ============================================================
        # Phase A
        # ============================================================
        out_sb = cpool.tile([P, spr], bf16, tag="out_sb")
        nmac = toka // cfg.a_mac
        msub = cfg.a_mac // P
        xTk = xT.rearrange("(k p) c -> p k c", k=kch)
        with tc.tile_pool(name="apool", bufs=2) as apl:
            for mblk in range(nmac):
                xt = apl.tile([P, kch * cfg.a_mac], bf16, tag="xt")
                DMA(
                    out=xt[:],
                    in_=xTk[:, :, mblk * cfg.a_mac:(mblk + 1) * cfg.a_mac])

                def lhsT_a(s, _xt=xt):
                    return [(_xt[:, k * cfg.a_mac + s * P:
                                 k * cfg.a_mac + (s + 1) * P], w1_sb[k])
                            for k in range(kch)]

                mlp_block(msub, lhsT_a, cst[:, 0:1], cst[:, 1:2], w2b_sb,
                          out_sb[:, mblk * msub:(mblk + 1) * msub])
        DMA(out=r2(out_part), in_=out_sb[:])

        CC("AllGather", Alu.bypass, replica_groups=groups,
           ins=[out_part[:]], outs=[out_tab[0:n_cyc]])
        DMA(out=r2(out_tab[n_cyc:n_cyc + ELEM], p=1), in_=sent0[:])
        fence()

        def stub_outputs():
            dumf = cpool.tile([P, fcols], f32, tag="dumf")
            nc.gpsimd.memset(dumf[:], 0.0)
            dume = cpool.tile([P, ecols], f32, tag="dume")
            nc.gpsimd.memset(dume[:], 0.0)
            DMA(out=r2(y_out), in_=dumf[:])
            DMA(out=r2(y2_out), in_=dume[:])

        if "b" not in cfg.phases:
            stub_outputs()
            return _finish(ctx)

        # ============================================================
        # Phase B
        # ============================================================
        with tc.tile_pool(name="bpool", bufs=2) as bpl, \
                tc.tile_pool(name="bstage", bufs=1) as bst:
            wstage = bst.tile([P, t1], f32, tag="wstage")
            ustage = bst.tile([P, t1], f32, tag="ustage")
            for i in range(nch_b):
                sl = slice(i * mcols, (i + 1) * mcols)
                val = gather_chunk(bpl, col(out_tab[:]), bpos[:, sl],
                                   mcols, "bg")
                bpc = bpl.tile([P, mcols], f32, tag="bpc")
                DMA(out=bpc[:], in_=bp[:, sl])
                bmc = bpl.tile([P, mcols], f32, tag="bmc")
                DMA(out=bmc[:], in_=bmsk[:, sl])
                stt(bpc[:], bpc[:], cfg.neg_slope, bpc[:], Alu.mult,
                    Alu.max)
                nc.scalar.activation(out=bpc[:], in_=bpc[:], func=Act.Exp)
                uval = bpl.tile([P, mcols], f32, tag="uval")
                nc.vector.tensor_tensor(out=uval[:], in0=bpc[:],
                                        in1=val[:], op=Alu.mult)
                nc.vector.tensor_tensor_scan(
                    out=wstage[:, sl], data0=bmc[:], data1=bpc[:],
                    initial=(0.0 if i == 0 else
                             wstage[:, i * mcols - 1:i * mcols]),
                    op0=Alu.mult, op1=Alu.add)
                nc.vector.tensor_tensor_scan(
                    out=ustage[:, sl], data0=bmc[:], data1=uval[:],
                    initial=(0.0 if i == 0 else
                             ustage[:, i * mcols - 1:i * mcols]),
                    op0=Alu.mult, op1=Alu.add)
            DMA(out=r2(bredW), in_=wstage[:])
            DMA(out=r2(bredU), in_=ustage[:])
            fence()

        # ---- segment extraction: dW/dU at end slots, then o2
        bidx_sb = cpool.tile([P, bcols], i32, tag="bidx_sb")
        DMA(out=bidx_sb[:], in_=bidx[:, :])
        dW = cpool.tile([P, bcols], f32, tag="dW")
        IDMA(out=dW[:], out_offset=None, in_=col(bredW[:]),
             in_offset=bass.IndirectOffsetOnAxis(ap=bidx_sb[:], axis=0))
        dU = cpool.tile([P, bcols], f32, tag="dU")
        IDMA(out=dU[:], out_offset=None, in_=col(bredU[:]),
             in_offset=bass.IndirectOffsetOnAxis(ap=bidx_sb[:], axis=0))
        o2 = cpool.tile([P, bcols], f32, tag="o2")
        nc.vector.tensor_scalar_add(out=o2[:], in0=dW[:], scalar1=1e-30)
        nc.vector.reciprocal(out=o2[:], in_=o2[:])
        nc.vector.tensor_tensor(out=o2[:], in0=o2[:], in1=dU[:],
                                op=Alu.mult)
        nc.vector.tensor_scalar_add(out=o2[:], in0=o2[:],
                                    scalar1=cfg.cshift)
        DMA(out=r2(out2_part), in_=o2[:])
        fence()
        CC("AllGather", Alu.bypass, replica_groups=groups,
           ins=[out2_part[:]], outs=[out2_tab[0:n_cyc]])
        DMA(out=r2(out2_tab[n_cyc:n_cyc + ELEM], p=1), in_=sent1[:])
        fence()

        if "c" not in cfg.phases:
            stub_outputs()
            return _finish(ctx)

        # ============================================================
        # Phase C
        # ============================================================
        with tc.tile_pool(name="cpool2", bufs=2) as cpl, \
                tc.tile_pool(name="cstage", bufs=1) as cstg:
            mstage = cstg.tile([P, t2], f32, tag="mstage")
            for i in range(nch_c):
                sl = slice(i * mcols, (i + 1) * mcols)
                val = gather_chunk(cpl, col(out2_tab[:]), cpos[:, sl],
                                   mcols, "cg", vdt=t2dt)
                cmc = cpl.tile([P, mcols], f32, tag="cmc")
                DMA(out=cmc[:], in_=cmsk[:, sl])
                if cfg.tab2_bf16:
                    valf = cpl.tile([P, mcols], f32, tag="valf")
                    nc.vector.tensor_copy(out=valf[:], in_=val[:])
                else:
                    valf = val
                nc.vector.tensor_tensor_scan(
                    out=mstage[:, sl], data0=cmc[:], data1=valf[:],
                    initial=(0.0 if i == 0 else
                             mstage[:, i * mcols - 1:i * mcols]),
                    op0=Alu.mult, op1=Alu.max)
            DMA(out=r2(credM), in_=mstage[:])
            fence()

        # ---- per-target max extraction (LOCAL) + om -> featT
        cidx_sb = cpool.tile([P, fcols], i32, tag="cidx_sb")
        DMA(out=cidx_sb[:], in_=cidxf[:, :])
        omr = cpool.tile([P, fcols], f32, tag="omr")
        IDMA(out=omr[:], out_offset=None, in_=col(credM[:]),
             in_offset=bass.IndirectOffsetOnAxis(ap=cidx_sb[:], axis=0))
        omm = cpool.tile([P, fcols], f32, tag="omm")
        nc.vector.tensor_scalar(out=omm[:], in0=omr[:], scalar1=32.0,
                                scalar2=None, op0=Alu.is_gt)
        om = cpool.tile([P, fcols], f32, tag="om")
        stt(om[:], omr[:], -cfg.cshift, omm[:], Alu.add, Alu.mult)
        DMA(out=r2(featT[0, :]), in_=om[:])
        wkb = cpool.tile([P, fcols], f32, tag="wkb")
        DMA(out=wkb[:], in_=r2(wkf))
        DMA(out=r2(featT[1, :]), in_=wkb[:])
        fence()

        if "d" not in cfg.phases:
            stub_outputs()
            return _finish(ctx)

        # ============================================================
        # Phase D: full MLP on own nonempty targets
        # ============================================================
        yF = cpool.tile([P, fcols], f32, tag="yF")
        nchunk = tokdf // cfg.d_chunk
        dsub = cfg.d_chunk // P
        with tc.tile_pool(name="dpool", bufs=2) as dpl:
            for ch in range(nchunk):
                ft = dpl.tile([2, cfg.d_chunk], f32, tag="ft")
                DMA(
                    out=ft[:],
                    in_=featT[:, ch * cfg.d_chunk:(ch + 1) * cfg.d_chunk])

                def lhsT_d(s, _ft=ft):
                    return [(_ft[:, s * P:(s + 1) * P], wk1_sb)]

                mlp_block(dsub, lhsT_d, cst[:, 2:3], cst[:, 3:4], wk2b_sb,
                          yF[:, ch * dsub:(ch + 1) * dsub])

        fence()

        # ---- fast path: empty targets (om == 0)
        wke_sb = cpool.tile([P, ecols], f32, tag="wke_sb")
        DMA(out=wke_sb[:], in_=wke[:, :])
        wk2t = cpool.tile([P, ecols], f32, tag="wk2t")
        stt(wk2t[:], wke_sb[:], 1.0, wke_sb[:], Alu.mult, Alu.mult)
        nc.vector.tensor_scalar(out=wk2t[:], in0=wk2t[:],
                                scalar1=cst[:, 5:6], scalar2=cst[:, 4:5],
                                op0=Alu.mult, op1=Alu.add)
        nc.scalar.activation(out=wk2t[:], in_=wk2t[:], func=Act.Sqrt)
        nc.vector.reciprocal(out=wk2t[:], in_=wk2t[:])
        yE = cpool.tile([P, ecols], f32, tag="yE")
        nc.vector.tensor_scalar(out=yE[:], in0=wke_sb[:],
                                scalar1=cst[:, 6:7], scalar2=None,
                                op0=Alu.mult)
        nc.vector.tensor_tensor(out=yE[:], in0=yE[:], in1=wk2t[:],
                                op=Alu.mult)
        nc.vector.tensor_scalar(out=yE[:], in0=yE[:], scalar1=cst[:, 3:4],
                                scalar2=None, op0=Alu.add)

        # ---- global L2 norm
        mf_sb = cpool.tile([P, fcols], f32, tag="mf_sb")
        DMA(out=mf_sb[:], in_=mf[:, :])
        me_sb = cpool.tile([P, ecols], f32, tag="me_sb")
        DMA(out=me_sb[:], in_=me[:, :])
        ssq = cpool.tile([P, 2], f32, tag="ssq")
        scrF = cpool.tile([P, fcols], f32, tag="scrF")
        nc.vector.tensor_tensor(out=scrF[:], in0=yF[:], in1=mf_sb[:],
                                op=Alu.mult)
        scrF2 = cpool.tile([P, fcols], f32, tag="scrF2")
        stt(scrF2[:], scrF[:], 1.0, yF[:], Alu.mult, Alu.mult,
            accum=ssq[:, 0:1])
        scrE = cpool.tile([P, ecols], f32, tag="scrE")
        nc.vector.tensor_tensor(out=scrE[:], in0=yE[:], in1=me_sb[:],
                                op=Alu.mult)
        scrE2 = cpool.tile([P, ecols], f32, tag="scrE2")
        stt(scrE2[:], scrE[:], 1.0, yE[:], Alu.mult, Alu.mult,
            accum=ssq[:, 1:2])
        ssqt = cpool.tile([P, 1], f32, tag="ssqt")
        nc.vector.tensor_tensor(out=ssqt[:], in0=ssq[:, 0:1],
                                in1=ssq[:, 1:2], op=Alu.add)
        ones = cpool.tile([P, 1], f32, tag="ones")
        nc.gpsimd.memset(ones[:], 1.0)
        sred = ps1.tile([1, 1], f32, tag="sred")
        nc.tensor.matmul(out=sred[:], lhsT=ones[:], rhs=ssqt[:],
                         start=True, stop=True)
        nsq_sb = cpool.tile([1, 16], f32, tag="nsq_sb")
        nc.gpsimd.memset(nsq_sb[:], 0.0)
        nc.vector.tensor_copy(out=nsq_sb[:, 0:1], in_=sred[:])
        DMA(out=r2(nsq_part, p=1), in_=nsq_sb[:])
        fence()
        CC("AllReduce", Alu.add, replica_groups=groups,
           ins=[nsq_part[:]], outs=[nsq_tab[:]])
        fence()
        nrm = cpool.tile([1, 1], f32, tag="nrm")
        DMA(out=nrm[:], in_=r2(nsq_tab[0:1], p=1))
        nc.scalar.activation(out=nrm[:], in_=nrm[:], func=Act.Sqrt)
        nc.vector.tensor_scalar_max(out=nrm[:], in0=nrm[:], scalar1=1e-12)
        nc.vector.reciprocal(out=nrm[:], in_=nrm[:])
        ones_row = cpool.tile([1, P], f32, tag="ones_row")
        nc.gpsimd.memset(ones_row[:], 1.0)
        rn_ps = ps1.tile([P, 1], f32, tag="rn_ps")
        nc.tensor.matmul(out=rn_ps[:], lhsT=ones_row[:], rhs=nrm[:],
                         start=True, stop=True)
        rn_sb = cpool.tile([P, 1], f32, tag="rn_sb")
        nc.vector.tensor_copy(out=rn_sb[:], in_=rn_ps[:])
        # sigmoid(x) = 1/(1+exp(-x)) via Exp + HW reciprocal
        def scale_sigmoid(t):
            nc.scalar.activation(out=t, in_=t, func=Act.Exp,
                                 scale=nrn_sb[:, 0:1])
            nc.vector.tensor_scalar_add(out=t, in0=t, scalar1=1.0)
            nc.vector.reciprocal(out=t, in_=t)

        nrn_sb = cpool.tile([P, 1], f32, tag="nrn_sb")
        nc.vector.tensor_scalar_mul(out=nrn_sb[:], in0=rn_sb[:],
                                    scalar1=-1.0)
        scale_sigmoid(yF[:])
        DMA(out=r2(y_out), in_=yF[:])
        scale_sigmoid(yE[:])
        DMA(out=r2(y2_out), in_=yE[:])

    return nc


# ---------------------------------------------------------------------------
# entry point
# ---------------------------------------------------------------------------

_NC_CACHE = {}


def _get_nc(cfg):
    key = (cfg.n_cyc, cfg.e_cc, cfg.len_edges, cfg.t1, cfg.t2,
           cfg.tokd_full, cfg.phases)
    if key not in _NC_CACHE:
        nc = build_nc(cfg)
        if not nc.is_finalized():
            nc.finalize()
        _NC_CACHE[key] = nc
    return _NC_CACHE[key]


def run(inputs, cfg=None, trace=False):
    from concourse.bass_utils import run_bass_kernel_spmd
    cfg = cfg or Cfg()
    in_maps, asm = host_prepare(inputs, cfg)
    nc = _get_nc(cfg)
    res = run_bass_kernel_spmd(nc, in_maps, core_ids=list(range(NCORES)),
                               trace=trace)
    return assemble_output(res.results, asm, cfg), res


def kernel(**inputs):
    out, _ = run(inputs)
    return out
